# revision 13
# baseline (speedup 1.0000x reference)
"""RNN-T Joint network kernel for 8x Trainium2 NeuronCores.

logits[b,t,u,v] = enc_out[b,t,:] @ W[v,:512] + pred_out[b,u,:] @ W[v,512:] + b[v]

Sharding: data-parallel over (B=4) x (T split in 2) -> 8 shards.
Core i handles b = i//2, t in [128*(i%2), 128*(i%2)+128).
Each core computes a contiguous (128, 64, 2048) f32 output slab (64 MB);
the kernel is bound by the HBM store stream (~64 MiB @ ~420 GB/s).

V-chunked streaming design (v in 4 chunks of 512): per chunk, load that
chunk's W rows (2 MiB, scalar-ring DMAs so they never queue behind output
stores on the sync ring), PE-transpose into W^T, matmul the tiny enc/pred
projections, then stream 8 stores of (128t, 8u, 512v) = 2 MiB each. Chunk
c+1's loads/transposes overlap chunk c's store stream, so the only
unhidden prologue is the first chunk's (~15 us) rather than a full
serial W load + transpose + project phase (~360 us in the old layout).

Main loop per 2 u's: broadcast pred_proj rows to 128 partitions with a
onehot matmul into PSUM (f32r stationary: 1 cyc/row), then one DVE add
(enc broadcast via stride-0 AP) -> SBUF out tile; store per 8 u's.
"""

import numpy as np

B, T, U = 4, 256, 64
D_ENC, D_PRED, VOCAB = 512, 512, 2048
D = D_ENC + D_PRED
TT = 128  # t rows per core
N_CORES = 8
NC = 4          # v chunks
VC = VOCAB // NC  # 512 v per chunk
GU = 8          # u's per store group

_cache = {}


def _build(reps=1):
    import concourse.bacc as bacc
    import concourse.mybir as mybir
    from concourse.tile import TileContext

    f32 = mybir.dt.float32
    f32r = mybir.dt.float32r

    nc = bacc.Bacc("TRN2", target_bir_lowering=False, debug=False, num_devices=N_CORES)
    if reps != 1:
        # bench-only: unused input whose shape encodes the bench config, so
        # the neuron compile cache (which doesn't hash the BIR) can't collide
        nc.dram_tensor("rep_marker", (1, reps), f32, kind="ExternalInput")
    enc_d = nc.dram_tensor("enc", (TT, D_ENC), f32, kind="ExternalInput")
    # identity | bias | 1.0  packed into one tensor -> one DMA -> one sem lane
    const_d = nc.dram_tensor("consts", (128, 128 + VOCAB + 1), f32, kind="ExternalInput")
    identr_d = nc.dram_tensor("identr", (128, 128), f32r, kind="ExternalInput")
    pred_d = nc.dram_tensor("pred", (U, D_PRED), f32, kind="ExternalInput")
    w_d = nc.dram_tensor("w", (VOCAB, D), f32, kind="ExternalInput")
    out_d = nc.dram_tensor("out", (TT, U, VOCAB), f32, kind="ExternalOutput")

    KD = D // 128   # 8 k-tiles along d
    KE = D_ENC // 128  # 4 enc k-tiles

    with TileContext(nc) as tc:
        with (
            tc.tile_pool(name="const", bufs=1) as const,
            tc.tile_pool(name="persist", bufs=1) as persist,
            tc.tile_pool(name="wraw", bufs=6) as wraw,
            tc.tile_pool(name="wt", bufs=2) as wt,
            tc.tile_pool(name="projc", bufs=2) as projc,
            tc.tile_pool(name="outp", bufs=3) as outp,
            tc.tile_pool(name="ps_tr", bufs=2, space="PSUM") as ps_tr,
            tc.tile_pool(name="ps_pj", bufs=2, space="PSUM") as ps_pj,
            tc.tile_pool(name="ps_m", bufs=2, space="PSUM") as ps_m,
        ):
            # consts layout: [:, :128] identity; [0, 128:128+V] bias; [0, -1] 1.0
            consts = const.tile([128, 128 + VOCAB + 1], f32)
            nc.scalar.dma_start(out=consts, in_=const_d[:])
            ident = consts[:, 0:128]
            bias_sb = consts[0:1, 128:128 + VOCAB]
            ones = consts[0:1, 128 + VOCAB:]
            # f32r identity: column u broadcast over 128 cols is the lhsT that
            # broadcasts pred_proj row u across 128 output partitions
            # (f32r: 1 cyc/row LDWEIGHTS); also the lhsT that accumulates
            # enc_proj into PSUM on the ScalarE-copy path.
            identr = const.tile([128, 128], f32r)
            nc.scalar.dma_start(out=identr, in_=identr_d[:])

            # PE pre-consumes each const DMA once (1 wait per inst) so no
            # later instruction needs >2 sync-wait commands (ISA limit).
            ps_dummy = ps_pj.tile([128, 512], f32, tag="ps")
            nc.tensor.transpose(ps_dummy[:, :128], ident, ident)
            nc.tensor.matmul(
                ps_dummy[:, :128], lhsT=identr, rhs=identr,
                start=True, stop=True,
            )

            # ---- enc^T (4 k-tiles of (128d, 128t)) and pred^T ((128d, 64u))
            enc_t = persist.tile([128, KE * TT], f32)
            pred_t = persist.tile([128, KE * U], f32)
            enc_sb = persist.tile([128, D_ENC], f32)
            nc.scalar.dma_start(out=enc_sb, in_=enc_d[:])
            pred_sb = persist.tile([U, D_PRED], f32)
            nc.scalar.dma_start(out=pred_sb, in_=pred_d[:])
            for k in range(KE):
                ps = ps_pj.tile([128, 512], f32, tag="ps")
                nc.tensor.transpose(
                    ps[:, :128], enc_sb[:, k * 128:(k + 1) * 128], ident)
                nc.tensor.transpose(
                    ps[:, 128:128 + U], pred_sb[:, k * 128:(k + 1) * 128],
                    ident[:U, :U])
                nc.scalar.copy(out=enc_t[:, k * TT:(k + 1) * TT], in_=ps[:, :128])
                nc.scalar.copy(out=pred_t[:, k * U:(k + 1) * U],
                               in_=ps[:, 128:128 + U])

            def _chunk(c):
                # -- load this chunk's 512 W rows as 4 tiles (scalar ring)
                raws = []
                for j in range(4):
                    w_raw = wraw.tile([128, D], f32, tag="w_raw")
                    nc.scalar.dma_start(
                        out=w_raw,
                        in_=w_d[c * VC + j * 128: c * VC + (j + 1) * 128, :])
                    raws.append(w_raw)
                # -- W^T chunk: (128d, 8k x 512v), copies split DVE/ScalarE
                w_tc = wt.tile([128, KD * VC], f32, tag="w_tc")
                for k in range(KD):
                    ps = ps_tr.tile([128, VC], f32, tag="ps")
                    for j in range(4):
                        nc.tensor.transpose(
                            ps[:, j * 128:(j + 1) * 128],
                            raws[j][:, k * 128:(k + 1) * 128], ident)
                    dst = w_tc[:, k * VC:(k + 1) * VC]
                    if k % 2 == 0:
                        nc.vector.tensor_copy(dst, ps)
                    else:
                        nc.scalar.copy(out=dst, in_=ps)
                # -- enc_proj chunk (128t, 512v)
                ps = ps_pj.tile([128, VC], f32, tag="ps")
                for k in range(KE):
                    nc.tensor.matmul(
                        ps, lhsT=enc_t[:, k * TT:(k + 1) * TT],
                        rhs=w_tc[:, k * VC:(k + 1) * VC],
                        start=(k == 0), stop=(k == KE - 1))
                enc_c = projc.tile([128, VC], f32r, tag="enc_c")
                nc.scalar.copy(out=enc_c, in_=ps)
                # -- pred_proj chunk (64u, 512v) + bias
                ps2 = ps_pj.tile([128, VC], f32, tag="ps")
                for k in range(KE):
                    nc.tensor.matmul(
                        ps2[:U], lhsT=pred_t[:, k * U:(k + 1) * U],
                        rhs=w_tc[:, (KE + k) * VC:(KE + k + 1) * VC],
                        start=(k == 0), stop=False)
                nc.tensor.matmul(
                    ps2[:U], lhsT=ones.broadcast_to((1, U)),
                    rhs=bias_sb[:, c * VC:(c + 1) * VC],
                    start=False, stop=True)
                pred_c = projc.tile([U, VC], f32r, tag="pred_c")
                nc.scalar.copy(out=pred_c, in_=ps2[:U])

                # -- main: 8 store groups of (128t, 8u, 512v) = 2 MiB.
                # Each PSUM tile covers 2 u's. 3 of 8 tiles take the ScalarE
                # path (PE also accumulates enc via identity-matmul, ScalarE
                # copies PSUM->SBUF); the rest take the DVE path (tensor_add
                # with stride-0-broadcast enc operand). Balances DVE/ACT.
                for g in range(U // GU):
                    o = outp.tile([128, GU * VC], f32, tag="o")
                    for h in range(GU // 2):
                        tile_idx = g * (GU // 2) + h
                        use_act = tile_idx % 8 < 3
                        ps = ps_m.tile([128, 2 * VC], f32, tag="ps")
                        for s in range(2):
                            u = g * GU + h * 2 + s
                            nc.tensor.matmul(
                                ps[:, s * VC:(s + 1) * VC],
                                lhsT=identr[:U, u:u + 1].broadcast_to((U, 128)),
                                rhs=pred_c, start=True, stop=not use_act)
                        dst = o[:, h * 2 * VC:(h + 1) * 2 * VC]
                        if use_act:
                            for s in range(2):
                                nc.tensor.matmul(
                                    ps[:, s * VC:(s + 1) * VC],
                                    lhsT=identr, rhs=enc_c,
                                    start=False, stop=True)
                            nc.scalar.copy(out=dst, in_=ps)
                        else:
                            nc.vector.tensor_add(
                                dst.rearrange("p (a v) -> p a v", a=2),
                                enc_c[:, None, :].broadcast_to((128, 2, VC)),
                                ps.rearrange("p (a v) -> p a v", a=2))
                    nc.sync.dma_start(
                        out=out_d[:, g * GU:(g + 1) * GU, c * VC:(c + 1) * VC],
                        in_=o.rearrange("p (a v) -> p a v", a=GU))

            for _rep in range(reps):
                for c in range(NC):
                    _chunk(c)
    nc.compile()
    return nc


def _make_in_maps(enc_out, pred_out, W, b):
    w_c = np.ascontiguousarray(W.astype(np.float32))
    consts = np.zeros((128, 128 + VOCAB + 1), dtype=np.float32)
    consts[:128, :128] = np.eye(128, dtype=np.float32)
    consts[0, 128:128 + VOCAB] = b.astype(np.float32)
    consts[0, -1] = 1.0
    ident = np.eye(128, dtype=np.float32)
    in_maps = []
    for i in range(N_CORES):
        bi, th = i // 2, i % 2
        in_maps.append({
            "enc": np.ascontiguousarray(enc_out[bi, th * TT:(th + 1) * TT, :].astype(np.float32)),
            "pred": np.ascontiguousarray(pred_out[bi].astype(np.float32)),
            "w": w_c,
            "consts": consts,
            "identr": ident,
        })
    return in_maps


def kernel(enc_out, pred_out, W, b):
    import os

    from concourse.bass_utils import run_bass_kernel_spmd

    if "nc" not in _cache:
        _cache["nc"] = _build()
    nc = _cache["nc"]
    trace = bool(os.environ.get("KJN_TRACE"))

    in_maps = _make_in_maps(enc_out, pred_out, W, b)

    kw = {}
    if trace:
        kw = dict(trace=True, trace_cores=[0], stitch_traces=False)
    res = run_bass_kernel_spmd(nc, in_maps, core_ids=list(range(N_CORES)), **kw)
    if trace:
        print(f"HW exec time: {res.exec_time_ns} ns")
        print(f"trace: {res.instructions_and_trace[1] if res.instructions_and_trace else None}")
        print(f"profile_json: {res.profile_json}")
    out = np.empty((B, T, U, VOCAB), dtype=np.float32)
    for i in range(N_CORES):
        bi, th = i // 2, i % 2
        out[bi, th * TT:(th + 1) * TT] = res.results[i]["out"]
    return out


# revision 14
# speedup vs baseline: 1.1342x; 1.1342x over previous
"""RNN-T Joint network kernel for 8x Trainium2 NeuronCores.

logits[b,t,u,v] = enc_out[b,t,:] @ W[v,:512] + pred_out[b,u,:] @ W[v,512:] + b[v]

Sharding: data-parallel over (B=4) x (T split in 2) -> 8 shards.
Core i handles b = i//2, t in [128*(i%2), 128*(i%2)+128).
Each core computes a contiguous (128, 64, 2048) f32 output slab (64 MB);
the kernel is bound by the HBM store stream (~64 MiB @ ~420 GB/s).

V-chunked streaming design (v in 4 chunks of 512): per chunk, load that
chunk's W rows (2 MiB, scalar-ring DMAs so they never queue behind output
stores on the sync ring), PE-transpose into W^T, matmul the tiny enc/pred
projections, then stream 8 stores of (128t, 8u, 512v) = 2 MiB each. Chunk
c+1's loads/transposes overlap chunk c's store stream, so the only
unhidden prologue is the first chunk's (~15 us) rather than a full
serial W load + transpose + project phase (~360 us in the old layout).

Main loop per 2 u's: broadcast pred_proj rows to 128 partitions with a
onehot matmul into PSUM (f32r stationary: 1 cyc/row), then one DVE add
(enc broadcast via stride-0 AP) -> SBUF out tile; store per 8 u's.
"""

import numpy as np

B, T, U = 4, 256, 64
D_ENC, D_PRED, VOCAB = 512, 512, 2048
D = D_ENC + D_PRED
TT = 128  # t rows per core
N_CORES = 8
NC = 4          # v chunks
VC = VOCAB // NC  # 512 v per chunk
GU = 8          # u's per store group

_cache = {}


def _build(reps=1, act_n=3):
    import concourse.bacc as bacc
    import concourse.mybir as mybir
    from concourse.tile import TileContext

    f32 = mybir.dt.float32
    f32r = mybir.dt.float32r

    nc = bacc.Bacc("TRN2", target_bir_lowering=False, debug=False, num_devices=N_CORES)
    if reps != 1:
        # bench-only: unused input whose shape encodes the bench config, so
        # the neuron compile cache (which doesn't hash the BIR) can't collide
        nc.dram_tensor("rep_marker", (1 + act_n, reps), f32, kind="ExternalInput")
    enc_d = nc.dram_tensor("enc", (TT, D_ENC), f32, kind="ExternalInput")
    # identity | bias | 1.0  packed into one tensor -> one DMA -> one sem lane
    const_d = nc.dram_tensor("consts", (128, 128 + VOCAB + 1), f32, kind="ExternalInput")
    identr_d = nc.dram_tensor("identr", (128, 128), f32r, kind="ExternalInput")
    pred_d = nc.dram_tensor("pred", (U, D_PRED), f32, kind="ExternalInput")
    w_d = nc.dram_tensor("w", (VOCAB, D), f32, kind="ExternalInput")
    out_d = nc.dram_tensor("out", (TT, U, VOCAB), f32, kind="ExternalOutput")

    KD = D // 128   # 8 k-tiles along d
    KE = D_ENC // 128  # 4 enc k-tiles

    with TileContext(nc) as tc:
        with (
            tc.tile_pool(name="const", bufs=1) as const,
            tc.tile_pool(name="persist", bufs=1) as persist,
            tc.tile_pool(name="wraw", bufs=6) as wraw,
            tc.tile_pool(name="wt", bufs=2) as wt,
            tc.tile_pool(name="projc", bufs=2) as projc,
            tc.tile_pool(name="outp", bufs=3) as outp,
            tc.tile_pool(name="ps_tr", bufs=2, space="PSUM") as ps_tr,
            tc.tile_pool(name="ps_pj", bufs=2, space="PSUM") as ps_pj,
            tc.tile_pool(name="ps_m", bufs=2, space="PSUM") as ps_m,
        ):
            # consts layout: [:, :128] identity; [0, 128:128+V] bias; [0, -1] 1.0
            consts = const.tile([128, 128 + VOCAB + 1], f32)
            nc.scalar.dma_start(out=consts, in_=const_d[:])
            ident = consts[:, 0:128]
            bias_sb = consts[0:1, 128:128 + VOCAB]
            ones = consts[0:1, 128 + VOCAB:]
            # f32r identity: column u broadcast over 128 cols is the lhsT that
            # broadcasts pred_proj row u across 128 output partitions
            # (f32r: 1 cyc/row LDWEIGHTS); also the lhsT that accumulates
            # enc_proj into PSUM on the ScalarE-copy path.
            identr = const.tile([128, 128], f32r)
            nc.scalar.dma_start(out=identr, in_=identr_d[:])

            # PE pre-consumes each const DMA once (1 wait per inst) so no
            # later instruction needs >2 sync-wait commands (ISA limit).
            ps_dummy = ps_pj.tile([128, 512], f32, tag="ps")
            nc.tensor.transpose(ps_dummy[:, :128], ident, ident)
            nc.tensor.matmul(
                ps_dummy[:, :128], lhsT=identr, rhs=identr,
                start=True, stop=True,
            )

            # ---- enc^T (4 k-tiles of (128d, 128t)) and pred^T ((128d, 64u))
            enc_t = persist.tile([128, KE * TT], f32)
            pred_t = persist.tile([128, KE * U], f32)
            enc_sb = persist.tile([128, D_ENC], f32)
            nc.scalar.dma_start(out=enc_sb, in_=enc_d[:])
            pred_sb = persist.tile([U, D_PRED], f32)
            nc.scalar.dma_start(out=pred_sb, in_=pred_d[:])
            for k in range(KE):
                ps = ps_pj.tile([128, 512], f32, tag="ps")
                nc.tensor.transpose(
                    ps[:, :128], enc_sb[:, k * 128:(k + 1) * 128], ident)
                nc.tensor.transpose(
                    ps[:, 128:128 + U], pred_sb[:, k * 128:(k + 1) * 128],
                    ident[:U, :U])
                nc.scalar.copy(out=enc_t[:, k * TT:(k + 1) * TT], in_=ps[:, :128])
                nc.scalar.copy(out=pred_t[:, k * U:(k + 1) * U],
                               in_=ps[:, 128:128 + U])

            def _chunk(c):
                # -- load this chunk's 512 W rows as 4 tiles (scalar ring)
                raws = []
                for j in range(4):
                    w_raw = wraw.tile([128, D], f32, tag="w_raw")
                    nc.scalar.dma_start(
                        out=w_raw,
                        in_=w_d[c * VC + j * 128: c * VC + (j + 1) * 128, :])
                    raws.append(w_raw)
                # -- W^T chunk: (128d, 8k x 512v), copies split DVE/ScalarE
                w_tc = wt.tile([128, KD * VC], f32, tag="w_tc")
                for k in range(KD):
                    ps = ps_tr.tile([128, VC], f32, tag="ps")
                    for j in range(4):
                        nc.tensor.transpose(
                            ps[:, j * 128:(j + 1) * 128],
                            raws[j][:, k * 128:(k + 1) * 128], ident)
                    dst = w_tc[:, k * VC:(k + 1) * VC]
                    if k % 2 == 0:
                        nc.vector.tensor_copy(dst, ps)
                    else:
                        nc.scalar.copy(out=dst, in_=ps)
                # -- enc_proj chunk (128t, 512v)
                ps = ps_pj.tile([128, VC], f32, tag="ps")
                for k in range(KE):
                    nc.tensor.matmul(
                        ps, lhsT=enc_t[:, k * TT:(k + 1) * TT],
                        rhs=w_tc[:, k * VC:(k + 1) * VC],
                        start=(k == 0), stop=(k == KE - 1))
                enc_c = projc.tile([128, VC], f32r, tag="enc_c")
                nc.scalar.copy(out=enc_c, in_=ps)
                # -- pred_proj chunk (64u, 512v) + bias
                ps2 = ps_pj.tile([128, VC], f32, tag="ps")
                for k in range(KE):
                    nc.tensor.matmul(
                        ps2[:U], lhsT=pred_t[:, k * U:(k + 1) * U],
                        rhs=w_tc[:, (KE + k) * VC:(KE + k + 1) * VC],
                        start=(k == 0), stop=False)
                nc.tensor.matmul(
                    ps2[:U], lhsT=ones.broadcast_to((1, U)),
                    rhs=bias_sb[:, c * VC:(c + 1) * VC],
                    start=False, stop=True)
                pred_c = projc.tile([U, VC], f32r, tag="pred_c")
                nc.scalar.copy(out=pred_c, in_=ps2[:U])

                # -- main: 8 store groups of (128t, 8u, 512v) = 2 MiB.
                # Each PSUM tile covers 2 u's. 3 of 8 tiles take the ScalarE
                # path (PE also accumulates enc via identity-matmul, ScalarE
                # copies PSUM->SBUF); the rest take the DVE path (tensor_add
                # with stride-0-broadcast enc operand). Balances DVE/ACT.
                for g in range(U // GU):
                    o = outp.tile([128, GU * VC], f32, tag="o")
                    for h in range(GU // 2):
                        tile_idx = g * (GU // 2) + h
                        use_act = tile_idx % 8 < act_n
                        ps = ps_m.tile([128, 2 * VC], f32, tag="ps")
                        for s in range(2):
                            u = g * GU + h * 2 + s
                            nc.tensor.matmul(
                                ps[:, s * VC:(s + 1) * VC],
                                lhsT=identr[:U, u:u + 1].broadcast_to((U, 128)),
                                rhs=pred_c, start=True, stop=not use_act)
                        dst = o[:, h * 2 * VC:(h + 1) * 2 * VC]
                        if use_act:
                            for s in range(2):
                                nc.tensor.matmul(
                                    ps[:, s * VC:(s + 1) * VC],
                                    lhsT=identr, rhs=enc_c,
                                    start=False, stop=True)
                            nc.scalar.copy(out=dst, in_=ps)
                        else:
                            nc.vector.tensor_add(
                                dst.rearrange("p (a v) -> p a v", a=2),
                                enc_c[:, None, :].broadcast_to((128, 2, VC)),
                                ps.rearrange("p (a v) -> p a v", a=2))
                    nc.sync.dma_start(
                        out=out_d[:, g * GU:(g + 1) * GU, c * VC:(c + 1) * VC],
                        in_=o.rearrange("p (a v) -> p a v", a=GU))

            for _rep in range(reps):
                for c in range(NC):
                    _chunk(c)
    nc.compile()
    return nc


def _make_in_maps(enc_out, pred_out, W, b):
    w_c = np.ascontiguousarray(W.astype(np.float32))
    consts = np.zeros((128, 128 + VOCAB + 1), dtype=np.float32)
    consts[:128, :128] = np.eye(128, dtype=np.float32)
    consts[0, 128:128 + VOCAB] = b.astype(np.float32)
    consts[0, -1] = 1.0
    ident = np.eye(128, dtype=np.float32)
    in_maps = []
    for i in range(N_CORES):
        bi, th = i // 2, i % 2
        in_maps.append({
            "enc": np.ascontiguousarray(enc_out[bi, th * TT:(th + 1) * TT, :].astype(np.float32)),
            "pred": np.ascontiguousarray(pred_out[bi].astype(np.float32)),
            "w": w_c,
            "consts": consts,
            "identr": ident,
        })
    return in_maps


def kernel(enc_out, pred_out, W, b):
    import os

    from concourse.bass_utils import run_bass_kernel_spmd

    if "nc" not in _cache:
        _cache["nc"] = _build()
    nc = _cache["nc"]
    trace = bool(os.environ.get("KJN_TRACE"))

    in_maps = _make_in_maps(enc_out, pred_out, W, b)

    kw = {}
    if trace:
        kw = dict(trace=True, trace_cores=[0], stitch_traces=False)
    res = run_bass_kernel_spmd(nc, in_maps, core_ids=list(range(N_CORES)), **kw)
    if trace:
        print(f"HW exec time: {res.exec_time_ns} ns")
        print(f"trace: {res.instructions_and_trace[1] if res.instructions_and_trace else None}")
        print(f"profile_json: {res.profile_json}")
    out = np.empty((B, T, U, VOCAB), dtype=np.float32)
    for i in range(N_CORES):
        bi, th = i // 2, i % 2
        out[bi, th * TT:(th + 1) * TT] = res.results[i]["out"]
    return out
